# revision 43
# baseline (speedup 1.0000x reference)
"""Trainium2 Bass kernel for multi-head cross-attention.

Problem: q [4, 2048, 512], kv [4, 4096, 128], 8 heads x 64 dim,
out = softmax((q Wq^T)(kv Wk^T)^T / 8) (kv Wv^T) Wo^T + b_o.

Sharding: pure data parallel over 8 NeuronCores; core c handles batch c//2,
query rows (c%2)*1024:(c%2+1)*1024. No collectives.

Per-core dataflow (all layouts feature-major so the PE contracts on partitions):
  - Inputs/weights: f32 HWDGE loads (sync+scalar rings in parallel) -> DVE
    cast to bf16 -> hardware xbar DMA transpose into feature-major SBUF.
  - Projections QT=[e,i], KT=[e,j], V=[j,e] via bf16 matmuls (fp32 PSUM).
  - A dependency-free 32-matmul warm-up burst right before the head loop:
    the PE clock gate (HAM) is bistable there -- warm entry self-sustains
    at 2.4 GHz, cold entry sticks at 1.2 GHz.
  - Per head (processed in base-partition pairs): S^T[j,i] matmuls (K=64);
    ACT exp with scale=1/8 folded into the activation (no max-subtract
    needed: logits ~ N(0,1), fp32 exp cannot overflow); AV matmuls use V
    augmented with a ones column so the softmax denominator falls out of
    the same accumulation (row 64 of the [65, i] PSUM accumulator).
  - Division by the denominator commutes past AV: one [65,i] copy frees
    the PSUM accumulator, then DRAM-roundtrip partition-broadcast of the
    denominator row, DVE approx-reciprocal, tensor_mul. Odd heads are
    relocated to base partition 64 via a DRAM roundtrip so out_linear can
    run K=128 matmuls over packed head pairs.
  - out_linear: 4 accumulating K=128 matmuls per 128-query chunk; bias
    (DMA partition-broadcast once) added on DVE; store.

Known-good HW numbers: ~430 us per core (8 cores run the same program
concurrently), rel err ~4.2e-3 vs the fp32 reference. Key scheduling facts:
PSUM (8 banks) is phase-split -- 4 single-bank projection slots released
before the two [65,1024] AV accumulators allocate; the head loop needs a
warm-entry matmul burst (HAM is bistable); the last pair's odd head feeds
out_linear directly from SBUF (K=64) to keep the DRAM roundtrip off the
tail critical path.
"""

import sys

import numpy as np

for _p in ("/opt/trn_rl_repo",):
    if _p not in sys.path:
        sys.path.insert(0, _p)

import concourse.bass as bass
import concourse.tile as tile
from concourse import mybir
from concourse.bass_utils import run_bass_kernel_spmd

F32 = mybir.dt.float32
BF16 = mybir.dt.bfloat16

B = 4
NQ_FULL = 2048
NQ = 1024          # queries per core
NK = 4096
C = 512            # q feature dim
KVC = 128          # kv feature dim
H = 8
D = 64
INNER = H * D      # 512
SCALE = D ** -0.5  # 0.125
EXPA = (2 ** 23 / float(np.log(2))) * SCALE   # Schraudolph slope (scale folded)
EXPB = 127 * 2 ** 23 - 367500                 # Schraudolph offset (tuned)
N_CORES = 8
DEBUG_DUMP = False


def _emit(tc, nc, q, kv, w_q, w_k, w_v, w_o, b_o, out):
    Exp = mybir.ActivationFunctionType.Exp

    with (
        tc.tile_pool(name="const", bufs=1) as const,
        tc.tile_pool(name="stage", bufs=4) as stage,
        tc.tile_pool(name="big", bufs=1) as big,
        tc.tile_pool(name="epool", bufs=6) as epool,
        tc.tile_pool(name="hdr", bufs=2) as hdr,
        tc.tile_pool(name="outp", bufs=2) as outp,
        tc.tile_pool(name="dscr", bufs=2, space="DRAM") as dscr,
        tc.tile_pool(name="ps", bufs=2, space="PSUM") as ps,
    ):
        # 4 single-bank PSUM slots for the projection phase; released before
        # the AV accumulators allocate (PSUM is only 8 banks)
        pre_ps = tc.alloc_tile_pool(name="preps", bufs=4, space="PSUM")
        # Layouts (feature-major so the PE contracts over partitions):
        wqT = big.tile([128, 4, C], BF16)      # [c_in, cb, e]
        wkT = big.tile([128, C], BF16)         # [c, e]
        wvT = big.tile([128, C], BF16)         # [c, e]
        qTin = big.tile([128, 4, NQ], BF16)    # [c_in, cb, i]
        kvT = big.tile([128, NK], BF16)        # [c, j]
        KT = big.tile([128, 4, NK], BF16)      # [e_in, et, j]
        QT = big.tile([128, 4, NQ], BF16)      # [e_in, et, i]
        vaug = big.tile([128, 32, H, 65], BF16)  # [j_in, jc, h, d|ones]
        woT128 = big.tile([128, 4, C], BF16)   # [e_in, hp, o]
        aoT128 = big.tile([128, 4, NQ], BF16)  # attn-out^T packed head pairs

        _cpidx = [0]
        _act_copies_ok = [True]

        def copy_ps(dst, src):
            # alternate PSUM->SBUF drains between DVE and ACT, but keep ACT
            # exclusively for exp once the head loop is running
            _cpidx[0] += 1
            if _act_copies_ok[0] and _cpidx[0] % 2 == 0:
                nc.scalar.copy(dst, src)
            else:
                nc.vector.tensor_copy(dst, src)

        def load_cast_transpose(src_ap, shape, dst_t, eng, teng, tagf, bufs=None):
            # f32 HWDGE load -> DVE cast to bf16 -> xbar DMA transpose
            t_f = stage.tile(shape, F32, tag=tagf, name="t_f", bufs=bufs)
            eng.dma_start(out=t_f, in_=src_ap)
            t_b = stage.tile(shape, BF16, tag=tagf + "b", name="t_b", bufs=1)
            nc.vector.tensor_copy(t_b, t_f)
            teng.dma_start_transpose(dst_t, t_b)

        def load_cast_transpose4(src4, dsts, eng, teng, tagf):
            # one 1MB f32 load of 4 row-blocks + one cast + 4 transposes
            t_f = stage.tile([128, 4, C], F32, tag=tagf, name="t_f", bufs=2)
            eng.dma_start(out=t_f, in_=src4.rearrange("(b p) c -> p b c",
                                                      p=128))
            t_b = stage.tile([128, 4, C], BF16, tag=tagf + "b", name="t_b",
                             bufs=2)
            nc.vector.tensor_copy(t_b, t_f)
            for b, dst in enumerate(dsts):
                teng.dma_start_transpose(dst, t_b[:, b, :])

        # ---- w_k (scalar ring) + kv (sync ring) in parallel ----
        load_cast_transpose(w_k.rearrange("(eb p) c -> p eb c", p=128),
                            [128, 4, KVC],
                            wkT.rearrange("p (eb e) -> p eb e", eb=4),
                            nc.scalar, nc.scalar, "wkv", bufs=1)
        for ch in range(2):
            load_cast_transpose(
                kv[ch * 2048:(ch + 1) * 2048, :].rearrange(
                    "(a p) d -> p a d", p=128),
                [128, 16, KVC],
                kvT[:, ch * 2048:(ch + 1) * 2048].rearrange(
                    "p (a j) -> p a j", a=16),
                nc.sync, nc.sync, "kvst", bufs=2)

        def emit_KT(et, jcs):
            for jc in jcs:
                kt_ps = pre_ps.tile([128, 512], F32, tag="px", name="kt_ps")
                nc.tensor.matmul(kt_ps,
                                 lhsT=wkT[:, et * 128:(et + 1) * 128],
                                 rhs=kvT[:, jc * 512:(jc + 1) * 512],
                                 start=True, stop=True)
                copy_ps(KT[:, et, jc * 512:(jc + 1) * 512], kt_ps)

        # small warm-up so the projection stream starts at speed
        warm0 = ps.tile([128, 512], F32, tag="sx", name="warm0")
        for _ in range(12):
            nc.tensor.matmul(warm0, lhsT=wkT[:, 0:128], rhs=kvT[:, 0:512],
                             start=True, stop=True)
        for _et in range(4):
            emit_KT(_et, range(8))

        # ---- q (scalar ring) + w_q (sync ring), then QT(et=0) ----
        for g in range(2):
            load_cast_transpose4(
                q[g * 512:(g + 1) * 512, :],
                [qTin[:, :, (g * 4 + b) * 128:(g * 4 + b + 1) * 128]
                 for b in range(4)],
                nc.scalar, nc.scalar, "stage4")
        load_cast_transpose4(
            w_q, [wqT[:, :, eb * 128:(eb + 1) * 128] for eb in range(4)],
            nc.sync, nc.sync, "stage4")

        def emit_QT(et, ihs):
            for ih in ihs:
                qt_ps = pre_ps.tile([128, 512], F32, tag="px", name="qt_ps")
                for cb in range(4):
                    nc.tensor.matmul(qt_ps,
                                     lhsT=wqT[:, cb, et * 128:(et + 1) * 128],
                                     rhs=qTin[:, cb, ih * 512:(ih + 1) * 512],
                                     start=(cb == 0), stop=(cb == 3))
                copy_ps(QT[:, et, ih * 512:(ih + 1) * 512], qt_ps)

        for _et in range(4):
            emit_QT(_et, range(2))

        # ---- w_v + V projection ----
        load_cast_transpose(w_v.rearrange("(eb p) c -> p eb c", p=128),
                            [128, 4, KVC],
                            wvT.rearrange("p (eb e) -> p eb e", eb=4),
                            nc.scalar, nc.scalar, "wkv", bufs=1)
        nc.vector.memset(vaug[:, :, :, 64:65], 1.0)

        def emit_V(jc):
            v_ps = pre_ps.tile([128, 512], F32, tag="px", name="v_ps")
            nc.tensor.matmul(v_ps,
                             lhsT=kvT[:, jc * 128:(jc + 1) * 128],
                             rhs=wvT,
                             start=True, stop=True)
            copy_ps(vaug[:, jc, :, 0:64],
                    v_ps.rearrange("p (h d) -> p h d", h=H))

        for _jc in range(32):
            emit_V(_jc)

        # ---- w_o + bias ----
        load_cast_transpose4(
            w_o, [woT128[:, :, ob * 128:(ob + 1) * 128] for ob in range(4)],
            nc.sync, nc.sync, "stage4")
        bias_bc = const.tile([128, C], F32)
        nc.gpsimd.dma_start(out=bias_bc, in_=b_o.to_broadcast([128, C]))
        # base-0 copies of the last pair's w_o halves: lets out_linear finish
        # heads 6/7 with K=64 matmuls, skipping the tail DRAM roundtrip
        woT64 = big.tile([64, 2, C], BF16)
        w6scr = dscr.tile([64, 2, C], BF16, name="w6scr")
        nc.gpsimd.dma_start(out=w6scr[:, 0, :], in_=woT128[0:64, 3, :])
        nc.gpsimd.dma_start(out=w6scr[:, 1, :], in_=woT128[64:128, 3, :])
        nc.gpsimd.dma_start(out=woT64, in_=w6scr)

        pre_ps.release()
        avps = tc.alloc_tile_pool(name="avps", bufs=2, space="PSUM")

        # warm-up burst: dependency-free back-to-back matmuls immediately
        # before the head loop so the PE enters it at 2.4 GHz (HAM is
        # bistable: warm entry self-sustains, cold entry sticks cold)
        warm_ps = ps.tile([128, 512], F32, tag="sx", name="warm_ps")
        for _ in range(32):
            nc.tensor.matmul(warm_ps, lhsT=wkT[:, 0:128], rhs=kvT[:, 0:512],
                             start=True, stop=True)

        # ---- attention: head pairs ----
        last_aodd = [None]

        def head_tail(hp, k, av_ps):
            # free the PSUM accumulator quickly with one [65, i] copy
            avsb = hdr.tile([65, NQ], F32, tag="avsb", name="avsb")
            nc.vector.tensor_copy(avsb, av_ps)
            rdram = dscr.tile([1, NQ], F32, name="rdram")
            nc.gpsimd.dma_start(out=rdram, in_=avsb[64:65, :])
            rbc = hdr.tile([64, NQ], F32, tag="rbc", name="rbc")
            nc.gpsimd.dma_start(out=rbc, in_=rdram.to_broadcast([64, NQ]))
            rbcr = hdr.tile([64, NQ], F32, tag="rbcr", name="rbcr")
            nc.vector.reciprocal_approx_fast(rbcr, rbc)
            if k == 0:
                nc.vector.tensor_mul(aoT128[0:64, hp, :], avsb[0:64, :], rbcr)
            else:
                aodd = hdr.tile([64, NQ], BF16, tag="aodd", name="aodd")
                nc.vector.tensor_mul(aodd, avsb[0:64, :], rbcr)
                if hp == 3:
                    last_aodd[0] = aodd
                else:
                    ascr = dscr.tile([64, NQ], BF16, name="ascr")
                    nc.gpsimd.dma_start(out=ascr, in_=aodd)
                    nc.gpsimd.dma_start(out=aoT128[64:128, hp, :], in_=ascr)

        _act_copies_ok[0] = False
        for hp in range(4):
            et = hp
            avs = [avps.tile([65, NQ], F32, name="av_ps") for _ in range(2)]
            for jc in range(32):
                sts = [ps.tile([128, NQ], F32, tag="sx", name="st_ps")
                       for _ in range(2)]
                # k-interleaved: adjacent matmuls use disjoint 64-row PE
                # groups (rows 0-63 vs 64-127) and overlap in the array
                for ih in range(2):
                    for k in range(2):
                        row = k * 64
                        nc.tensor.matmul(
                            sts[k][:, ih * 512:(ih + 1) * 512],
                            lhsT=KT[row:row + 64, et, jc * 128:(jc + 1) * 128],
                            rhs=QT[row:row + 64, et, ih * 512:(ih + 1) * 512],
                            start=True, stop=True)
                exps = []
                for k in range(2):
                    expS = epool.tile([128, NQ], BF16, name="expS")
                    nc.scalar.activation(out=expS, in_=sts[k], func=Exp,
                                         scale=SCALE)
                    exps.append(expS)
                for k in range(2):
                    for ih in range(2):
                        nc.tensor.matmul(
                            avs[k][:, ih * 512:(ih + 1) * 512],
                            lhsT=vaug[:, jc, 2 * hp + k, :],
                            rhs=exps[k][:, ih * 512:(ih + 1) * 512],
                            start=(jc == 0), stop=(jc == 31))
            for k in range(2):
                head_tail(hp, k, avs[k])

        if DEBUG_DUMP:
            for nm, t in [("kvT", kvT), ("wkT", wkT), ("wvT", wvT),
                          ("qTin", qTin), ("wqT", wqT), ("KT", KT),
                          ("QT", QT), ("vaug", vaug), ("aoT128", aoT128),
                          ("woT128", woT128)]:
                dd = nc.dram_tensor("d_" + nm, list(t.shape), t.dtype,
                                    kind="ExternalOutput").ap()
                nc.gpsimd.dma_start(out=dd, in_=t)

        # ---- out linear: K=128 over pairs 0-2, K=64 for heads 6/7 ----
        for ic in range(8):
            ol_ps = ps.tile([128, 512], F32, tag="sx", name="ol_ps")
            for hp in range(3):
                nc.tensor.matmul(ol_ps,
                                 lhsT=aoT128[:, hp, ic * 128:(ic + 1) * 128],
                                 rhs=woT128[:, hp, :],
                                 start=(hp == 0), stop=False)
            nc.tensor.matmul(ol_ps,
                             lhsT=aoT128[0:64, 3, ic * 128:(ic + 1) * 128],
                             rhs=woT64[:, 0, :], start=False, stop=False)
            nc.tensor.matmul(ol_ps,
                             lhsT=last_aodd[0][:, ic * 128:(ic + 1) * 128],
                             rhs=woT64[:, 1, :], start=False, stop=True)
            o_sb = outp.tile([128, C], F32, tag="osb", name="o_sb")
            if ic % 2:
                # ACT is idle in the tail; split the bias adds across engines
                nc.scalar.activation(out=o_sb, in_=ol_ps,
                                     func=mybir.ActivationFunctionType.Copy,
                                     bias=0.0)
                nc.vector.tensor_add(o_sb, o_sb, bias_bc)
            else:
                nc.vector.tensor_add(o_sb, ol_ps, bias_bc)
            eng = nc.sync if ic % 2 else nc.scalar
            eng.dma_start(out=out[ic * 128:(ic + 1) * 128, :], in_=o_sb)
        avps.release()


def build_program():
    from concourse import bacc
    nc = bacc.Bacc("TRN2", target_bir_lowering=False, debug=False)
    q = nc.dram_tensor("q", [NQ, C], F32, kind="ExternalInput").ap()
    kv = nc.dram_tensor("kv", [NK, KVC], F32, kind="ExternalInput").ap()
    w_q = nc.dram_tensor("w_q", [INNER, C], F32, kind="ExternalInput").ap()
    w_k = nc.dram_tensor("w_k", [INNER, KVC], F32, kind="ExternalInput").ap()
    w_v = nc.dram_tensor("w_v", [INNER, KVC], F32, kind="ExternalInput").ap()
    w_o = nc.dram_tensor("w_o", [C, INNER], F32, kind="ExternalInput").ap()
    b_o = nc.dram_tensor("b_o", [1, C], F32, kind="ExternalInput").ap()
    out = nc.dram_tensor("out", [NQ, C], F32, kind="ExternalOutput").ap()
    with tile.TileContext(nc) as tc:
        _emit(tc, nc, q, kv, w_q, w_k, w_v, w_o, b_o, out)
    nc.compile()
    return nc


def make_in_maps(q, kv, w_q, w_k, w_v, w_o, b_o):
    q = np.ascontiguousarray(q, dtype=np.float32)
    kv = np.ascontiguousarray(kv, dtype=np.float32)
    w_q = np.ascontiguousarray(w_q, dtype=np.float32)
    w_k = np.ascontiguousarray(w_k, dtype=np.float32)
    w_v = np.ascontiguousarray(w_v, dtype=np.float32)
    w_o = np.ascontiguousarray(w_o, dtype=np.float32)
    b_o = np.ascontiguousarray(b_o, dtype=np.float32).reshape(1, C)
    in_maps = []
    for core in range(N_CORES):
        b, half = core // 2, core % 2
        in_maps.append({
            "q": np.ascontiguousarray(q[b, half * NQ:(half + 1) * NQ]),
            "kv": kv[b],
            "w_q": w_q, "w_k": w_k, "w_v": w_v, "w_o": w_o, "b_o": b_o,
        })
    return in_maps


def assemble(results):
    out = np.zeros((B, NQ_FULL, C), np.float32)
    for core in range(N_CORES):
        b, half = core // 2, core % 2
        out[b, half * NQ:(half + 1) * NQ] = results[core]["out"]
    return out


def run(inputs, trace=False, **kwargs):
    nc = build_program()
    in_maps = make_in_maps(**inputs)
    res = run_bass_kernel_spmd(nc, in_maps, core_ids=list(range(N_CORES)),
                               trace=trace, **kwargs)
    return assemble(res.results), res


def kernel(q, kv, w_q, w_k, w_v, w_o, b_o):
    out, _ = run(dict(q=q, kv=kv, w_q=w_q, w_k=w_k, w_v=w_v, w_o=w_o, b_o=b_o))
    return out


# revision 44
# speedup vs baseline: 1.0037x; 1.0037x over previous
"""Trainium2 Bass kernel for multi-head cross-attention.

Problem: q [4, 2048, 512], kv [4, 4096, 128], 8 heads x 64 dim,
out = softmax((q Wq^T)(kv Wk^T)^T / 8) (kv Wv^T) Wo^T + b_o.

Sharding: pure data parallel over 8 NeuronCores; core c handles batch c//2,
query rows (c%2)*1024:(c%2+1)*1024. No collectives.

Per-core dataflow (all layouts feature-major so the PE contracts on partitions):
  - Inputs/weights: f32 HWDGE loads (sync+scalar rings in parallel) -> DVE
    cast to bf16 -> hardware xbar DMA transpose into feature-major SBUF.
  - Projections QT=[e,i], KT=[e,j], V=[j,e] via bf16 matmuls (fp32 PSUM).
  - A dependency-free 32-matmul warm-up burst right before the head loop:
    the PE clock gate (HAM) is bistable there -- warm entry self-sustains
    at 2.4 GHz, cold entry sticks at 1.2 GHz.
  - Per head (processed in base-partition pairs): S^T[j,i] matmuls (K=64);
    ACT exp with scale=1/8 folded into the activation (no max-subtract
    needed: logits ~ N(0,1), fp32 exp cannot overflow); AV matmuls use V
    augmented with a ones column so the softmax denominator falls out of
    the same accumulation (row 64 of the [65, i] PSUM accumulator).
  - Division by the denominator commutes past AV: one [65,i] copy frees
    the PSUM accumulator, then DRAM-roundtrip partition-broadcast of the
    denominator row, DVE approx-reciprocal, tensor_mul. Odd heads are
    relocated to base partition 64 via a DRAM roundtrip so out_linear can
    run K=128 matmuls over packed head pairs.
  - out_linear: 4 accumulating K=128 matmuls per 128-query chunk; bias
    (DMA partition-broadcast once) added on DVE; store.

Known-good HW numbers: ~430 us per core (8 cores run the same program
concurrently), rel err ~4.2e-3 vs the fp32 reference. Key scheduling facts:
PSUM (8 banks) is phase-split -- 4 single-bank projection slots released
before the two [65,1024] AV accumulators allocate; the head loop needs a
warm-entry matmul burst (HAM is bistable); the last pair's odd head feeds
out_linear directly from SBUF (K=64) to keep the DRAM roundtrip off the
tail critical path.
"""

import sys

import numpy as np

for _p in ("/opt/trn_rl_repo",):
    if _p not in sys.path:
        sys.path.insert(0, _p)

import concourse.bass as bass
import concourse.tile as tile
from concourse import mybir
from concourse.bass_utils import run_bass_kernel_spmd

F32 = mybir.dt.float32
BF16 = mybir.dt.bfloat16

B = 4
NQ_FULL = 2048
NQ = 1024          # queries per core
NK = 4096
C = 512            # q feature dim
KVC = 128          # kv feature dim
H = 8
D = 64
INNER = H * D      # 512
SCALE = D ** -0.5  # 0.125
EXPA = (2 ** 23 / float(np.log(2))) * SCALE   # Schraudolph slope (scale folded)
EXPB = 127 * 2 ** 23 - 367500                 # Schraudolph offset (tuned)
N_CORES = 8
DEBUG_DUMP = False


def _emit(tc, nc, q, kv, w_q, w_k, w_v, w_o, b_o, out):
    Exp = mybir.ActivationFunctionType.Exp

    with (
        tc.tile_pool(name="const", bufs=1) as const,
        tc.tile_pool(name="stage", bufs=4) as stage,
        tc.tile_pool(name="big", bufs=1) as big,
        tc.tile_pool(name="epool", bufs=6) as epool,
        tc.tile_pool(name="hdr", bufs=2) as hdr,
        tc.tile_pool(name="outp", bufs=2) as outp,
        tc.tile_pool(name="dscr", bufs=2, space="DRAM") as dscr,
        tc.tile_pool(name="ps", bufs=2, space="PSUM") as ps,
    ):
        # 4 single-bank PSUM slots for the projection phase; released before
        # the AV accumulators allocate (PSUM is only 8 banks)
        pre_ps = tc.alloc_tile_pool(name="preps", bufs=4, space="PSUM")
        # Layouts (feature-major so the PE contracts over partitions):
        wqT = big.tile([128, 4, C], BF16)      # [c_in, cb, e]
        wkT = big.tile([128, C], BF16)         # [c, e]
        wvT = big.tile([128, C], BF16)         # [c, e]
        qTin = big.tile([128, 4, NQ], BF16)    # [c_in, cb, i]
        kvT = big.tile([128, NK], BF16)        # [c, j]
        KT = big.tile([128, 4, NK], BF16)      # [e_in, et, j]
        QT = big.tile([128, 4, NQ], BF16)      # [e_in, et, i]
        vaug = big.tile([128, 32, H, 65], BF16)  # [j_in, jc, h, d|ones]
        woT128 = big.tile([128, 4, C], BF16)   # [e_in, hp, o]
        aoT128 = big.tile([128, 4, NQ], BF16)  # attn-out^T packed head pairs

        _cpidx = [0]
        _act_copies_ok = [True]

        def copy_ps(dst, src):
            # alternate PSUM->SBUF drains between DVE and ACT, but keep ACT
            # exclusively for exp once the head loop is running
            _cpidx[0] += 1
            if _act_copies_ok[0] and _cpidx[0] % 2 == 0:
                nc.scalar.copy(dst, src)
            else:
                nc.vector.tensor_copy(dst, src)

        def load_cast_transpose(src_ap, shape, dst_t, eng, teng, tagf, bufs=None):
            # f32 HWDGE load -> DVE cast to bf16 -> xbar DMA transpose
            t_f = stage.tile(shape, F32, tag=tagf, name="t_f", bufs=bufs)
            eng.dma_start(out=t_f, in_=src_ap)
            t_b = stage.tile(shape, BF16, tag=tagf + "b", name="t_b", bufs=1)
            nc.vector.tensor_copy(t_b, t_f)
            teng.dma_start_transpose(dst_t, t_b)

        def load_cast_transpose4(src4, dsts, eng, teng, tagf):
            # one 1MB f32 load of 4 row-blocks + one cast + 4 transposes
            t_f = stage.tile([128, 4, C], F32, tag=tagf, name="t_f", bufs=2)
            eng.dma_start(out=t_f, in_=src4.rearrange("(b p) c -> p b c",
                                                      p=128))
            t_b = stage.tile([128, 4, C], BF16, tag=tagf + "b", name="t_b",
                             bufs=2)
            nc.vector.tensor_copy(t_b, t_f)
            for b, dst in enumerate(dsts):
                teng.dma_start_transpose(dst, t_b[:, b, :])

        # ---- w_k (scalar ring) + kv (sync ring) in parallel ----
        load_cast_transpose(w_k.rearrange("(eb p) c -> p eb c", p=128),
                            [128, 4, KVC],
                            wkT.rearrange("p (eb e) -> p eb e", eb=4),
                            nc.scalar, nc.scalar, "wkv", bufs=1)
        for ch in range(2):
            load_cast_transpose(
                kv[ch * 2048:(ch + 1) * 2048, :].rearrange(
                    "(a p) d -> p a d", p=128),
                [128, 16, KVC],
                kvT[:, ch * 2048:(ch + 1) * 2048].rearrange(
                    "p (a j) -> p a j", a=16),
                nc.sync, nc.sync, "kvst", bufs=2)

        def emit_KT(et, jcs):
            for jc in jcs:
                kt_ps = pre_ps.tile([128, 512], F32, tag="px", name="kt_ps")
                nc.tensor.matmul(kt_ps,
                                 lhsT=wkT[:, et * 128:(et + 1) * 128],
                                 rhs=kvT[:, jc * 512:(jc + 1) * 512],
                                 start=True, stop=True)
                copy_ps(KT[:, et, jc * 512:(jc + 1) * 512], kt_ps)

        # small warm-up so the projection stream starts at speed
        warm0 = ps.tile([128, 512], F32, tag="sx", name="warm0")
        for _ in range(12):
            nc.tensor.matmul(warm0, lhsT=wkT[:, 0:128], rhs=kvT[:, 0:512],
                             start=True, stop=True)
        for _et in range(4):
            emit_KT(_et, range(8))

        # ---- q (scalar ring) + w_q (sync ring), then QT(et=0) ----
        for g in range(2):
            load_cast_transpose4(
                q[g * 512:(g + 1) * 512, :],
                [qTin[:, :, (g * 4 + b) * 128:(g * 4 + b + 1) * 128]
                 for b in range(4)],
                nc.scalar, nc.scalar, "stage4")
        load_cast_transpose4(
            w_q, [wqT[:, :, eb * 128:(eb + 1) * 128] for eb in range(4)],
            nc.sync, nc.sync, "stage4")

        def emit_QT(et, ihs):
            for ih in ihs:
                qt_ps = pre_ps.tile([128, 512], F32, tag="px", name="qt_ps")
                for cb in range(4):
                    nc.tensor.matmul(qt_ps,
                                     lhsT=wqT[:, cb, et * 128:(et + 1) * 128],
                                     rhs=qTin[:, cb, ih * 512:(ih + 1) * 512],
                                     start=(cb == 0), stop=(cb == 3))
                copy_ps(QT[:, et, ih * 512:(ih + 1) * 512], qt_ps)

        for _et in range(4):
            emit_QT(_et, range(2))

        # ---- w_v + V projection ----
        load_cast_transpose(w_v.rearrange("(eb p) c -> p eb c", p=128),
                            [128, 4, KVC],
                            wvT.rearrange("p (eb e) -> p eb e", eb=4),
                            nc.scalar, nc.scalar, "wkv", bufs=1)
        nc.vector.memset(vaug[:, :, :, 64:65], 1.0)

        def emit_V(jc):
            v_ps = pre_ps.tile([128, 512], F32, tag="px", name="v_ps")
            nc.tensor.matmul(v_ps,
                             lhsT=kvT[:, jc * 128:(jc + 1) * 128],
                             rhs=wvT,
                             start=True, stop=True)
            copy_ps(vaug[:, jc, :, 0:64],
                    v_ps.rearrange("p (h d) -> p h d", h=H))

        for _jc in range(32):
            emit_V(_jc)

        # ---- w_o + bias ----
        load_cast_transpose4(
            w_o, [woT128[:, :, ob * 128:(ob + 1) * 128] for ob in range(4)],
            nc.sync, nc.sync, "stage4")
        bias_bc = const.tile([128, C], F32)
        nc.gpsimd.dma_start(out=bias_bc, in_=b_o.to_broadcast([128, C]))
        # base-0 copies of the last pair's w_o halves: lets out_linear finish
        # heads 6/7 with K=64 matmuls, skipping the tail DRAM roundtrip
        woT64 = big.tile([64, 2, C], BF16)
        w6scr = dscr.tile([64, 2, C], BF16, name="w6scr")
        nc.gpsimd.dma_start(out=w6scr[:, 0, :], in_=woT128[0:64, 3, :])
        nc.gpsimd.dma_start(out=w6scr[:, 1, :], in_=woT128[64:128, 3, :])
        nc.gpsimd.dma_start(out=woT64, in_=w6scr)

        pre_ps.release()
        avps = tc.alloc_tile_pool(name="avps", bufs=2, space="PSUM")

        # warm-up burst: dependency-free back-to-back matmuls immediately
        # before the head loop so the PE enters it at 2.4 GHz (HAM is
        # bistable: warm entry self-sustains, cold entry sticks cold)
        warm_ps = ps.tile([128, 512], F32, tag="sx", name="warm_ps")
        for _ in range(32):
            nc.tensor.matmul(warm_ps, lhsT=wkT[:, 0:128], rhs=kvT[:, 0:512],
                             start=True, stop=True)

        # ---- attention: head pairs ----
        last_aodd = [None]

        def head_tail(hp, k, av_ps):
            # free the PSUM accumulator quickly with one [65, i] copy
            avsb = hdr.tile([65, NQ], F32, tag="avsb", name="avsb")
            nc.vector.tensor_copy(avsb, av_ps)
            rdram = dscr.tile([1, NQ], F32, name="rdram")
            nc.gpsimd.dma_start(out=rdram, in_=avsb[64:65, :])
            rbc = hdr.tile([64, NQ], F32, tag="rbc", name="rbc")
            nc.gpsimd.dma_start(out=rbc, in_=rdram.to_broadcast([64, NQ]))
            rbcr = hdr.tile([64, NQ], F32, tag="rbcr", name="rbcr")
            nc.vector.reciprocal_approx_fast(rbcr, rbc)
            if k == 0:
                nc.vector.tensor_mul(aoT128[0:64, hp, :], avsb[0:64, :], rbcr)
            else:
                aodd = hdr.tile([64, NQ], BF16, tag="aodd", name="aodd")
                nc.vector.tensor_mul(aodd, avsb[0:64, :], rbcr)
                if hp == 3:
                    last_aodd[0] = aodd
                else:
                    ascr = dscr.tile([64, NQ], BF16, name="ascr")
                    nc.gpsimd.dma_start(out=ascr, in_=aodd)
                    nc.gpsimd.dma_start(out=aoT128[64:128, hp, :], in_=ascr)

        _act_copies_ok[0] = False
        for hp in range(4):
            et = hp
            avs = [avps.tile([65, NQ], F32, name="av_ps") for _ in range(2)]
            for jc in range(32):
                sts = [ps.tile([128, NQ], F32, tag="sx", name="st_ps")
                       for _ in range(2)]
                # k-interleaved: adjacent matmuls use disjoint 64-row PE
                # groups (rows 0-63 vs 64-127) and overlap in the array
                for ih in range(2):
                    for k in range(2):
                        row = k * 64
                        nc.tensor.matmul(
                            sts[k][:, ih * 512:(ih + 1) * 512],
                            lhsT=KT[row:row + 64, et, jc * 128:(jc + 1) * 128],
                            rhs=QT[row:row + 64, et, ih * 512:(ih + 1) * 512],
                            start=True, stop=True)
                exps = []
                for k in range(2):
                    expS = epool.tile([128, NQ], BF16, name="expS")
                    nc.scalar.activation(out=expS, in_=sts[k], func=Exp,
                                         scale=SCALE)
                    exps.append(expS)
                for k in range(2):
                    for ih in range(2):
                        nc.tensor.matmul(
                            avs[k][:, ih * 512:(ih + 1) * 512],
                            lhsT=vaug[:, jc, 2 * hp + k, :],
                            rhs=exps[k][:, ih * 512:(ih + 1) * 512],
                            start=(jc == 0), stop=(jc == 31))
            for k in range(2):
                head_tail(hp, k, avs[k])

        if DEBUG_DUMP:
            for nm, t in [("kvT", kvT), ("wkT", wkT), ("wvT", wvT),
                          ("qTin", qTin), ("wqT", wqT), ("KT", KT),
                          ("QT", QT), ("vaug", vaug), ("aoT128", aoT128),
                          ("woT128", woT128)]:
                dd = nc.dram_tensor("d_" + nm, list(t.shape), t.dtype,
                                    kind="ExternalOutput").ap()
                nc.gpsimd.dma_start(out=dd, in_=t)

        # ---- out linear: K=128 over pairs 0-2, K=64 for heads 6/7 ----
        for ic in range(8):
            ol_ps = ps.tile([128, 512], F32, tag="sx", name="ol_ps")
            for hp in range(3):
                nc.tensor.matmul(ol_ps,
                                 lhsT=aoT128[:, hp, ic * 128:(ic + 1) * 128],
                                 rhs=woT128[:, hp, :],
                                 start=(hp == 0), stop=False)
            nc.tensor.matmul(ol_ps,
                             lhsT=aoT128[0:64, 3, ic * 128:(ic + 1) * 128],
                             rhs=woT64[:, 0, :], start=False, stop=False)
            nc.tensor.matmul(ol_ps,
                             lhsT=last_aodd[0][:, ic * 128:(ic + 1) * 128],
                             rhs=woT64[:, 1, :], start=False, stop=True)
            o_sb = outp.tile([128, C], F32, tag="osb", name="o_sb")
            nc.vector.tensor_add(o_sb, ol_ps, bias_bc)
            eng = nc.sync if ic % 2 else nc.scalar
            eng.dma_start(out=out[ic * 128:(ic + 1) * 128, :], in_=o_sb)
        avps.release()


def build_program():
    from concourse import bacc
    nc = bacc.Bacc("TRN2", target_bir_lowering=False, debug=False)
    q = nc.dram_tensor("q", [NQ, C], F32, kind="ExternalInput").ap()
    kv = nc.dram_tensor("kv", [NK, KVC], F32, kind="ExternalInput").ap()
    w_q = nc.dram_tensor("w_q", [INNER, C], F32, kind="ExternalInput").ap()
    w_k = nc.dram_tensor("w_k", [INNER, KVC], F32, kind="ExternalInput").ap()
    w_v = nc.dram_tensor("w_v", [INNER, KVC], F32, kind="ExternalInput").ap()
    w_o = nc.dram_tensor("w_o", [C, INNER], F32, kind="ExternalInput").ap()
    b_o = nc.dram_tensor("b_o", [1, C], F32, kind="ExternalInput").ap()
    out = nc.dram_tensor("out", [NQ, C], F32, kind="ExternalOutput").ap()
    with tile.TileContext(nc) as tc:
        _emit(tc, nc, q, kv, w_q, w_k, w_v, w_o, b_o, out)
    nc.compile()
    return nc


def make_in_maps(q, kv, w_q, w_k, w_v, w_o, b_o):
    q = np.ascontiguousarray(q, dtype=np.float32)
    kv = np.ascontiguousarray(kv, dtype=np.float32)
    w_q = np.ascontiguousarray(w_q, dtype=np.float32)
    w_k = np.ascontiguousarray(w_k, dtype=np.float32)
    w_v = np.ascontiguousarray(w_v, dtype=np.float32)
    w_o = np.ascontiguousarray(w_o, dtype=np.float32)
    b_o = np.ascontiguousarray(b_o, dtype=np.float32).reshape(1, C)
    in_maps = []
    for core in range(N_CORES):
        b, half = core // 2, core % 2
        in_maps.append({
            "q": np.ascontiguousarray(q[b, half * NQ:(half + 1) * NQ]),
            "kv": kv[b],
            "w_q": w_q, "w_k": w_k, "w_v": w_v, "w_o": w_o, "b_o": b_o,
        })
    return in_maps


def assemble(results):
    out = np.zeros((B, NQ_FULL, C), np.float32)
    for core in range(N_CORES):
        b, half = core // 2, core % 2
        out[b, half * NQ:(half + 1) * NQ] = results[core]["out"]
    return out


def run(inputs, trace=False, **kwargs):
    nc = build_program()
    in_maps = make_in_maps(**inputs)
    res = run_bass_kernel_spmd(nc, in_maps, core_ids=list(range(N_CORES)),
                               trace=trace, **kwargs)
    return assemble(res.results), res


def kernel(q, kv, w_q, w_k, w_v, w_o, b_o):
    out, _ = run(dict(q=q, kv=kv, w_q=w_q, w_k=w_k, w_v=w_v, w_o=w_o, b_o=b_o))
    return out


# revision 46
# speedup vs baseline: 1.0141x; 1.0104x over previous
"""Trainium2 Bass kernel for multi-head cross-attention.

Problem: q [4, 2048, 512], kv [4, 4096, 128], 8 heads x 64 dim,
out = softmax((q Wq^T)(kv Wk^T)^T / 8) (kv Wv^T) Wo^T + b_o.

Sharding: pure data parallel over 8 NeuronCores; core c handles batch c//2,
query rows (c%2)*1024:(c%2+1)*1024. No collectives.

Per-core dataflow (all layouts feature-major so the PE contracts on partitions):
  - Inputs/weights: f32 HWDGE loads (sync+scalar rings in parallel) -> DVE
    cast to bf16 -> hardware xbar DMA transpose into feature-major SBUF.
  - Projections QT=[e,i], KT=[e,j], V=[j,e] via bf16 matmuls (fp32 PSUM).
  - A dependency-free 32-matmul warm-up burst right before the head loop:
    the PE clock gate (HAM) is bistable there -- warm entry self-sustains
    at 2.4 GHz, cold entry sticks at 1.2 GHz.
  - Per head (processed in base-partition pairs): S^T[j,i] matmuls (K=64);
    ACT exp with scale=1/8 folded into the activation (no max-subtract
    needed: logits ~ N(0,1), fp32 exp cannot overflow); AV matmuls use V
    augmented with a ones column so the softmax denominator falls out of
    the same accumulation (row 64 of the [65, i] PSUM accumulator).
  - Division by the denominator commutes past AV: one [65,i] copy frees
    the PSUM accumulator, then DRAM-roundtrip partition-broadcast of the
    denominator row, DVE approx-reciprocal, tensor_mul. Odd heads are
    relocated to base partition 64 via a DRAM roundtrip so out_linear can
    run K=128 matmuls over packed head pairs.
  - out_linear: 4 accumulating K=128 matmuls per 128-query chunk; bias
    (DMA partition-broadcast once) added on DVE; store.

Known-good HW numbers: ~427 us per core (8 cores run the same program
concurrently), rel err ~4.2e-3 vs the fp32 reference. Key scheduling facts:
PSUM (8 banks) is phase-split -- 4 single-bank projection slots released
before the two [65,1024] AV accumulators allocate; the head loop needs a
warm-entry matmul burst (HAM is bistable); the last pair's odd head feeds
out_linear directly from SBUF (K=64) to keep the DRAM roundtrip off the
tail critical path.
"""

import sys

import numpy as np

for _p in ("/opt/trn_rl_repo",):
    if _p not in sys.path:
        sys.path.insert(0, _p)

import concourse.bass as bass
import concourse.tile as tile
from concourse import mybir
from concourse.bass_utils import run_bass_kernel_spmd

F32 = mybir.dt.float32
BF16 = mybir.dt.bfloat16

B = 4
NQ_FULL = 2048
NQ = 1024          # queries per core
NK = 4096
C = 512            # q feature dim
KVC = 128          # kv feature dim
H = 8
D = 64
INNER = H * D      # 512
SCALE = D ** -0.5  # 0.125
EXPA = (2 ** 23 / float(np.log(2))) * SCALE   # Schraudolph slope (scale folded)
EXPB = 127 * 2 ** 23 - 367500                 # Schraudolph offset (tuned)
N_CORES = 8
DEBUG_DUMP = False


def _emit(tc, nc, q, kv, w_q, w_k, w_v, w_o, b_o, out):
    Exp = mybir.ActivationFunctionType.Exp

    with (
        tc.tile_pool(name="const", bufs=1) as const,
        tc.tile_pool(name="stage", bufs=4) as stage,
        tc.tile_pool(name="big", bufs=1) as big,
        tc.tile_pool(name="epool", bufs=6) as epool,
        tc.tile_pool(name="hdr", bufs=2) as hdr,
        tc.tile_pool(name="outp", bufs=2) as outp,
        tc.tile_pool(name="dscr", bufs=2, space="DRAM") as dscr,
        tc.tile_pool(name="ps", bufs=2, space="PSUM") as ps,
    ):
        # 4 single-bank PSUM slots for the projection phase; released before
        # the AV accumulators allocate (PSUM is only 8 banks)
        pre_ps = tc.alloc_tile_pool(name="preps", bufs=4, space="PSUM")
        # Layouts (feature-major so the PE contracts over partitions):
        wqT = big.tile([128, 4, C], BF16)      # [c_in, cb, e]
        wkT = big.tile([128, C], BF16)         # [c, e]
        wvT = big.tile([128, C], BF16)         # [c, e]
        qTin = big.tile([128, 4, NQ], BF16)    # [c_in, cb, i]
        kvT = big.tile([128, NK], BF16)        # [c, j]
        KT = big.tile([128, 4, NK], BF16)      # [e_in, et, j]
        QT = big.tile([128, 4, NQ], BF16)      # [e_in, et, i]
        vaug = big.tile([128, 32, H, 65], BF16)  # [j_in, jc, h, d|ones]
        woT128 = big.tile([128, 4, C], BF16)   # [e_in, hp, o]
        aoT128 = big.tile([128, 4, NQ], BF16)  # attn-out^T packed head pairs

        _cpidx = [0]
        _act_copies_ok = [True]

        def copy_ps(dst, src):
            # alternate PSUM->SBUF drains between DVE and ACT, but keep ACT
            # exclusively for exp once the head loop is running
            _cpidx[0] += 1
            if _act_copies_ok[0] and _cpidx[0] % 2 == 0:
                nc.scalar.copy(dst, src)
            else:
                nc.vector.tensor_copy(dst, src)

        def load_cast_transpose(src_ap, shape, dst_t, eng, teng, tagf, bufs=None):
            # f32 HWDGE load -> DVE cast to bf16 -> xbar DMA transpose
            t_f = stage.tile(shape, F32, tag=tagf, name="t_f", bufs=bufs)
            eng.dma_start(out=t_f, in_=src_ap)
            t_b = stage.tile(shape, BF16, tag=tagf + "b", name="t_b", bufs=1)
            nc.vector.tensor_copy(t_b, t_f)
            teng.dma_start_transpose(dst_t, t_b)

        def load_cast_transpose4(src4, dsts, eng, teng, tagf):
            # one 1MB f32 load of 4 row-blocks + one cast + 4 transposes
            t_f = stage.tile([128, 4, C], F32, tag=tagf, name="t_f", bufs=2)
            eng.dma_start(out=t_f, in_=src4.rearrange("(b p) c -> p b c",
                                                      p=128))
            t_b = stage.tile([128, 4, C], BF16, tag=tagf + "b", name="t_b",
                             bufs=2)
            nc.vector.tensor_copy(t_b, t_f)
            for b, dst in enumerate(dsts):
                teng.dma_start_transpose(dst, t_b[:, b, :])

        # ---- w_k (scalar ring) + kv (sync ring) in parallel ----
        load_cast_transpose(w_k.rearrange("(eb p) c -> p eb c", p=128),
                            [128, 4, KVC],
                            wkT.rearrange("p (eb e) -> p eb e", eb=4),
                            nc.scalar, nc.scalar, "wkv", bufs=1)
        # contiguous kv reads (8KB/partition): rows land j-PERMUTED
        # (slot a*128+p <-> row 16p+a), which is harmless -- softmax reduces
        # over j and j never reaches the output, so no unpermute is needed
        for ch in range(2):
            load_cast_transpose(
                kv[ch * 2048:(ch + 1) * 2048, :].rearrange(
                    "(p a) d -> p a d", a=16),
                [128, 16, KVC],
                kvT[:, ch * 2048:(ch + 1) * 2048].rearrange(
                    "p (a j) -> p a j", a=16),
                nc.sync, nc.sync, "kvst", bufs=2)

        def emit_KT(et, jcs):
            for jc in jcs:
                kt_ps = pre_ps.tile([128, 512], F32, tag="px", name="kt_ps")
                nc.tensor.matmul(kt_ps,
                                 lhsT=wkT[:, et * 128:(et + 1) * 128],
                                 rhs=kvT[:, jc * 512:(jc + 1) * 512],
                                 start=True, stop=True)
                copy_ps(KT[:, et, jc * 512:(jc + 1) * 512], kt_ps)

        # small warm-up so the projection stream starts at speed
        warm0 = ps.tile([128, 512], F32, tag="sx", name="warm0")
        for _ in range(12):
            nc.tensor.matmul(warm0, lhsT=wkT[:, 0:128], rhs=kvT[:, 0:512],
                             start=True, stop=True)
        for _et in range(4):
            emit_KT(_et, range(8))

        # ---- q (scalar ring) + w_q (sync ring), then QT(et=0) ----
        for g in range(2):
            load_cast_transpose4(
                q[g * 512:(g + 1) * 512, :],
                [qTin[:, :, (g * 4 + b) * 128:(g * 4 + b + 1) * 128]
                 for b in range(4)],
                nc.scalar, nc.scalar, "stage4")
        load_cast_transpose4(
            w_q, [wqT[:, :, eb * 128:(eb + 1) * 128] for eb in range(4)],
            nc.sync, nc.sync, "stage4")

        def emit_QT(et, ihs):
            for ih in ihs:
                qt_ps = pre_ps.tile([128, 512], F32, tag="px", name="qt_ps")
                for cb in range(4):
                    nc.tensor.matmul(qt_ps,
                                     lhsT=wqT[:, cb, et * 128:(et + 1) * 128],
                                     rhs=qTin[:, cb, ih * 512:(ih + 1) * 512],
                                     start=(cb == 0), stop=(cb == 3))
                copy_ps(QT[:, et, ih * 512:(ih + 1) * 512], qt_ps)

        for _et in range(4):
            emit_QT(_et, range(2))

        # ---- w_v + V projection ----
        load_cast_transpose(w_v.rearrange("(eb p) c -> p eb c", p=128),
                            [128, 4, KVC],
                            wvT.rearrange("p (eb e) -> p eb e", eb=4),
                            nc.scalar, nc.scalar, "wkv", bufs=1)
        nc.vector.memset(vaug[:, :, :, 64:65], 1.0)

        def emit_V(jc):
            v_ps = pre_ps.tile([128, 512], F32, tag="px", name="v_ps")
            nc.tensor.matmul(v_ps,
                             lhsT=kvT[:, jc * 128:(jc + 1) * 128],
                             rhs=wvT,
                             start=True, stop=True)
            copy_ps(vaug[:, jc, :, 0:64],
                    v_ps.rearrange("p (h d) -> p h d", h=H))

        for _jc in range(32):
            emit_V(_jc)

        # ---- w_o + bias ----
        load_cast_transpose4(
            w_o, [woT128[:, :, ob * 128:(ob + 1) * 128] for ob in range(4)],
            nc.sync, nc.sync, "stage4")
        bias_bc = const.tile([128, C], F32)
        nc.gpsimd.dma_start(out=bias_bc, in_=b_o.to_broadcast([128, C]))
        # base-0 copies of the last pair's w_o halves: lets out_linear finish
        # heads 6/7 with K=64 matmuls, skipping the tail DRAM roundtrip
        woT64 = big.tile([64, 2, C], BF16)
        w6scr = dscr.tile([64, 2, C], BF16, name="w6scr")
        nc.gpsimd.dma_start(out=w6scr[:, 0, :], in_=woT128[0:64, 3, :])
        nc.gpsimd.dma_start(out=w6scr[:, 1, :], in_=woT128[64:128, 3, :])
        nc.gpsimd.dma_start(out=woT64, in_=w6scr)

        pre_ps.release()
        avps = tc.alloc_tile_pool(name="avps", bufs=2, space="PSUM")

        # warm-up burst: dependency-free back-to-back matmuls immediately
        # before the head loop so the PE enters it at 2.4 GHz (HAM is
        # bistable: warm entry self-sustains, cold entry sticks cold)
        warm_ps = ps.tile([128, 512], F32, tag="sx", name="warm_ps")
        for _ in range(32):
            nc.tensor.matmul(warm_ps, lhsT=wkT[:, 0:128], rhs=kvT[:, 0:512],
                             start=True, stop=True)

        # ---- attention: head pairs ----
        last_aodd = [None]

        def head_tail(hp, k, av_ps):
            # free the PSUM accumulator quickly with one [65, i] copy
            avsb = hdr.tile([65, NQ], F32, tag="avsb", name="avsb")
            nc.vector.tensor_copy(avsb, av_ps)
            rdram = dscr.tile([1, NQ], F32, name="rdram")
            nc.gpsimd.dma_start(out=rdram, in_=avsb[64:65, :])
            rbc = hdr.tile([64, NQ], F32, tag="rbc", name="rbc")
            nc.gpsimd.dma_start(out=rbc, in_=rdram.to_broadcast([64, NQ]))
            rbcr = hdr.tile([64, NQ], F32, tag="rbcr", name="rbcr")
            nc.vector.reciprocal_approx_fast(rbcr, rbc)
            if k == 0:
                nc.vector.tensor_mul(aoT128[0:64, hp, :], avsb[0:64, :], rbcr)
            else:
                aodd = hdr.tile([64, NQ], BF16, tag="aodd", name="aodd")
                nc.vector.tensor_mul(aodd, avsb[0:64, :], rbcr)
                if hp == 3:
                    last_aodd[0] = aodd
                else:
                    ascr = dscr.tile([64, NQ], BF16, name="ascr")
                    nc.gpsimd.dma_start(out=ascr, in_=aodd)
                    nc.gpsimd.dma_start(out=aoT128[64:128, hp, :], in_=ascr)

        _act_copies_ok[0] = False
        for hp in range(4):
            et = hp
            avs = [avps.tile([65, NQ], F32, name="av_ps") for _ in range(2)]
            for jc in range(32):
                sts = [ps.tile([128, NQ], F32, tag="sx", name="st_ps")
                       for _ in range(2)]
                # k-interleaved: adjacent matmuls use disjoint 64-row PE
                # groups (rows 0-63 vs 64-127) and overlap in the array
                for ih in range(2):
                    for k in range(2):
                        row = k * 64
                        nc.tensor.matmul(
                            sts[k][:, ih * 512:(ih + 1) * 512],
                            lhsT=KT[row:row + 64, et, jc * 128:(jc + 1) * 128],
                            rhs=QT[row:row + 64, et, ih * 512:(ih + 1) * 512],
                            start=True, stop=True)
                exps = []
                for k in range(2):
                    expS = epool.tile([128, NQ], BF16, name="expS")
                    nc.scalar.activation(out=expS, in_=sts[k], func=Exp,
                                         scale=SCALE)
                    exps.append(expS)
                for k in range(2):
                    for ih in range(2):
                        nc.tensor.matmul(
                            avs[k][:, ih * 512:(ih + 1) * 512],
                            lhsT=vaug[:, jc, 2 * hp + k, :],
                            rhs=exps[k][:, ih * 512:(ih + 1) * 512],
                            start=(jc == 0), stop=(jc == 31))
            for k in range(2):
                head_tail(hp, k, avs[k])

        if DEBUG_DUMP:
            for nm, t in [("kvT", kvT), ("wkT", wkT), ("wvT", wvT),
                          ("qTin", qTin), ("wqT", wqT), ("KT", KT),
                          ("QT", QT), ("vaug", vaug), ("aoT128", aoT128),
                          ("woT128", woT128)]:
                dd = nc.dram_tensor("d_" + nm, list(t.shape), t.dtype,
                                    kind="ExternalOutput").ap()
                nc.gpsimd.dma_start(out=dd, in_=t)

        # ---- out linear: K=128 over pairs 0-2, K=64 for heads 6/7 ----
        for ic in range(8):
            ol_ps = ps.tile([128, 512], F32, tag="sx", name="ol_ps")
            for hp in range(3):
                nc.tensor.matmul(ol_ps,
                                 lhsT=aoT128[:, hp, ic * 128:(ic + 1) * 128],
                                 rhs=woT128[:, hp, :],
                                 start=(hp == 0), stop=False)
            nc.tensor.matmul(ol_ps,
                             lhsT=aoT128[0:64, 3, ic * 128:(ic + 1) * 128],
                             rhs=woT64[:, 0, :], start=False, stop=False)
            nc.tensor.matmul(ol_ps,
                             lhsT=last_aodd[0][:, ic * 128:(ic + 1) * 128],
                             rhs=woT64[:, 1, :], start=False, stop=True)
            o_sb = outp.tile([128, C], F32, tag="osb", name="o_sb")
            nc.vector.tensor_add(o_sb, ol_ps, bias_bc)
            eng = nc.sync if ic % 2 else nc.scalar
            eng.dma_start(out=out[ic * 128:(ic + 1) * 128, :], in_=o_sb)
        avps.release()


def build_program():
    from concourse import bacc
    nc = bacc.Bacc("TRN2", target_bir_lowering=False, debug=False)
    q = nc.dram_tensor("q", [NQ, C], F32, kind="ExternalInput").ap()
    kv = nc.dram_tensor("kv", [NK, KVC], F32, kind="ExternalInput").ap()
    w_q = nc.dram_tensor("w_q", [INNER, C], F32, kind="ExternalInput").ap()
    w_k = nc.dram_tensor("w_k", [INNER, KVC], F32, kind="ExternalInput").ap()
    w_v = nc.dram_tensor("w_v", [INNER, KVC], F32, kind="ExternalInput").ap()
    w_o = nc.dram_tensor("w_o", [C, INNER], F32, kind="ExternalInput").ap()
    b_o = nc.dram_tensor("b_o", [1, C], F32, kind="ExternalInput").ap()
    out = nc.dram_tensor("out", [NQ, C], F32, kind="ExternalOutput").ap()
    with tile.TileContext(nc) as tc:
        _emit(tc, nc, q, kv, w_q, w_k, w_v, w_o, b_o, out)
    nc.compile()
    return nc


def make_in_maps(q, kv, w_q, w_k, w_v, w_o, b_o):
    q = np.ascontiguousarray(q, dtype=np.float32)
    kv = np.ascontiguousarray(kv, dtype=np.float32)
    w_q = np.ascontiguousarray(w_q, dtype=np.float32)
    w_k = np.ascontiguousarray(w_k, dtype=np.float32)
    w_v = np.ascontiguousarray(w_v, dtype=np.float32)
    w_o = np.ascontiguousarray(w_o, dtype=np.float32)
    b_o = np.ascontiguousarray(b_o, dtype=np.float32).reshape(1, C)
    in_maps = []
    for core in range(N_CORES):
        b, half = core // 2, core % 2
        in_maps.append({
            "q": np.ascontiguousarray(q[b, half * NQ:(half + 1) * NQ]),
            "kv": kv[b],
            "w_q": w_q, "w_k": w_k, "w_v": w_v, "w_o": w_o, "b_o": b_o,
        })
    return in_maps


def assemble(results):
    out = np.zeros((B, NQ_FULL, C), np.float32)
    for core in range(N_CORES):
        b, half = core // 2, core % 2
        out[b, half * NQ:(half + 1) * NQ] = results[core]["out"]
    return out


def run(inputs, trace=False, **kwargs):
    nc = build_program()
    in_maps = make_in_maps(**inputs)
    res = run_bass_kernel_spmd(nc, in_maps, core_ids=list(range(N_CORES)),
                               trace=trace, **kwargs)
    return assemble(res.results), res


def kernel(q, kv, w_q, w_k, w_v, w_o, b_o):
    out, _ = run(dict(q=q, kv=kv, w_q=w_q, w_k=w_k, w_v=w_v, w_o=w_o, b_o=b_o))
    return out
